# revision 53
# baseline (speedup 1.0000x reference)
"""AGT layer (GAT-style attention + relational bias + residual LayerNorm) on 8 TRN2 cores.

Sharding: 8 cores = 2 batches x 4 sequence-quarters. Each core computes the
full fr = h[b] @ Wr for its batch (redundant across the 4 quarter-cores, but
cheap) and then produces its own 512 output rows end-to-end with ZERO
collectives: bias scores for its rows, softmax, context, fh, residual+LN.

Algebraic simplifications (exact):
  - sl[i] (from Wl/al) is constant per softmax row -> softmax shift-invariance
    makes Wl/al/fl dead. Skipped entirely.
  - softmax denominator folded into the context matmul via a ones-column in
    the stationary operand.
  - sr[j] folded into the exp via ScalarE's per-partition bias operand.

Numerics: matmuls in bf16 (output is dominated by the f32 residual h; the
attention branch contributes ~0.005 sigma, so bf16 error lands ~1e-4 rel).
Scores are O(0.1) so exp without max-subtraction is safe.
"""

import sys
import numpy as np

sys.path.insert(0, "/opt/trn_rl_repo")

import ml_dtypes
from concourse import bacc, mybir, tile
from concourse.bass_utils import run_bass_kernel_spmd

BF16 = ml_dtypes.bfloat16
F32 = mybir.dt.float32
BF = mybir.dt.bfloat16

B, N, D = 2, 2048, 512
H, HD, RD = 8, 64, 16
SLOPE, EPS = 0.01, 1e-5
NCORE = 8
Q = 4            # sequence quarters per batch
RN = N // Q      # 512 rows owned per core
JC = N // 128    # 16 key-side chunks
IC = RN // 128   # 4 row-blocks per core
DC = D // 128    # 4 contraction chunks

_CACHE = {}


def _build_graph():
    nc = bacc.Bacc("TRN2", target_bir_lowering=False, debug=False,
                   num_devices=NCORE)

    # ---- per-core DRAM parameters (host supplies per-core shards) ----
    hT_d = nc.declare_dram_parameter("hT", [D, N], BF, isOutput=False)
    hrow_d = nc.declare_dram_parameter("hrow", [RN, D], F32, isOutput=False)
    rhT_d = nc.declare_dram_parameter("rhT", [RD, N], BF, isOutput=False)
    rhTq_d = nc.declare_dram_parameter("rhTq", [RD, RN], BF, isOutput=False)
    wr_d = nc.declare_dram_parameter("Wr", [D, D], BF, isOutput=False)
    wf_d = nc.declare_dram_parameter("Wf", [D, D], BF, isOutput=False)
    wrs_d = nc.declare_dram_parameter("Wrs", [RD, 3, 96], BF, isOutput=False)
    wrt_d = nc.declare_dram_parameter("Wrt", [RD, 3, 96], BF, isOutput=False)
    ar_d = nc.declare_dram_parameter("arT", [128, D], BF, isOutput=False)
    gam_d = nc.declare_dram_parameter("gamma", [128, D], F32, isOutput=False)
    bet_d = nc.declare_dram_parameter("beta", [128, D], F32, isOutput=False)
    out_d = nc.declare_dram_parameter("out", [RN, D], F32, isOutput=True)

    EXP = mybir.ActivationFunctionType.Exp
    LRELU = mybir.ActivationFunctionType.Lrelu
    SQRT = mybir.ActivationFunctionType.Sqrt
    COPY = mybir.ActivationFunctionType.Copy

    with tile.TileContext(nc) as tc:
        with (
            tc.tile_pool(name="const", bufs=1) as const,
            tc.tile_pool(name="pers", bufs=1) as pers,
            tc.tile_pool(name="work", bufs=3) as work,
            tc.tile_pool(name="atp", bufs=12) as atp,
            tc.tile_pool(name="fin", bufs=2) as fin,
            tc.tile_pool(name="ps", bufs=2, space="PSUM") as ps,
            tc.tile_pool(name="psfr", bufs=2, space="PSUM") as psfr,
            tc.tile_pool(name="psctx", bufs=4, space="PSUM") as psctx,
        ):
            # ---------- tiles ----------
            hT = const.tile([128, DC, N], BF)      # h[b].T, chunked on d
            wr = const.tile([128, DC, D], BF)
            wf = const.tile([128, DC, D], BF)
            hrow = const.tile([128, IC, D], F32)   # own rows of h (residual)
            rhT = const.tile([RD, N], BF)
            rhTq = const.tile([RD, RN], BF)
            wrs = const.tile([RD, 3, 96], BF)
            wrt = const.tile([RD, 3, 96], BF)
            arT = const.tile([128, D], BF)
            gam = const.tile([128, D], F32)
            bet = const.tile([128, D], F32)
            epsc = const.tile([128, 1], F32)
            ones64 = const.tile([1, HD], F32)

            fr = pers.tile([128, JC, H, HD + 1], BF)  # col HD = ones (denom)
            sr = pers.tile([128, JC, H], F32)         # per-key attn bias
            rq = pers.tile([96, 3, N], BF)   # group g rows 32*(h%3)..+16
            rk = pers.tile([96, 3, RN], BF)
            ctxT = pers.tile([128, DC, RN], BF)       # h_sa.T (head pairs)

            # ---------- DMAs (earliest-needed first) ----------
            nc.sync.dma_start(hT[:, 0, :], hT_d[0:128, :])
            nc.sync.dma_start(wr[:, 0, :], wr_d[0:128, :])
            nc.sync.dma_start(rhT[:], rhT_d[:])
            nc.sync.dma_start(rhTq[:], rhTq_d[:])
            nc.sync.dma_start(wrs[:], wrs_d[:])
            nc.sync.dma_start(wrt[:], wrt_d[:])
            nc.sync.dma_start(arT[:], ar_d[:])
            for c in range(1, DC):
                nc.sync.dma_start(hT[:, c, :], hT_d[c * 128:(c + 1) * 128, :])
                nc.sync.dma_start(wr[:, c, :], wr_d[c * 128:(c + 1) * 128, :])
            for c in range(DC):
                nc.sync.dma_start(wf[:, c, :], wf_d[c * 128:(c + 1) * 128, :])
            for c in range(IC):
                nc.sync.dma_start(hrow[:, c, :], hrow_d[c * 128:(c + 1) * 128, :])
            nc.sync.dma_start(gam[:], gam_d[:])
            nc.sync.dma_start(bet[:], bet_d[:])

            nc.vector.memset(fr[:, :, :, HD], 1.0)
            nc.vector.memset(epsc[:], EPS)
            nc.vector.memset(ones64[:], 1.0)

            # ---- rq / rk: 3 head-groups, heads at 32-aligned partitions ----
            def rq_chunk(g, c):
                rqp = ps.tile([96, 512], F32, tag="pp")
                nc.tensor.matmul(rqp[:], wrt[:, g, :],
                                 rhT[:, c * 512:(c + 1) * 512],
                                 start=True, stop=True)
                nc.scalar.activation(rq[:, g, c * 512:(c + 1) * 512],
                                     rqp[:], COPY)

            def rk_group(g):
                rkp = ps.tile([96, RN], F32, tag="pp")
                nc.tensor.matmul(rkp[:], wrs[:, g, :], rhTq[:],
                                 start=True, stop=True)
                nc.scalar.activation(rk[:, g, :], rkp[:], COPY)

            def rq_chunk_dve(g, c):
                rqp = ps.tile([96, 512], F32, tag="pp")
                nc.tensor.matmul(rqp[:], wrt[:, g, :],
                                 rhT[:, c * 512:(c + 1) * 512],
                                 start=True, stop=True)
                nc.vector.tensor_copy(rq[:, g, c * 512:(c + 1) * 512], rqp[:])

            def rk_group_dve(g):
                rkp = ps.tile([96, RN], F32, tag="pp")
                nc.tensor.matmul(rkp[:], wrs[:, g, :], rhTq[:],
                                 start=True, stop=True)
                nc.vector.tensor_copy(rk[:, g, :], rkp[:])

            # urgent: chunk 0 + rk of the two group-A groups (ACT, idle now)
            rq_chunk(0, 0)
            rk_group(0)
            rq_chunk(1, 0)
            rk_group(1)
            # deferred: remaining chunks fill the DMA-bound startup window
            for c in range(1, N // 512):
                rq_chunk(0, c)
                rq_chunk(1, c)
            for c in range(N // 512):
                rq_chunk(2, c)
            rk_group(2)


            HG = 4  # group A heads, inline with fr stream

            def bias_exp(h, jc):
                bp = ps.tile([128, RN], F32, tag="pp")
                g, o = h // 3, (h % 3) * 32
                nc.tensor.matmul(bp[:], rq[o:o + RD, g,
                                           jc * 128:(jc + 1) * 128],
                                 rk[o:o + RD, g, :],
                                 start=True, stop=True)
                at = atp.tile([128, RN], BF, tag="attn")
                nc.scalar.activation(at[:], bp[:], EXP,
                                     bias=sr[:, jc, h:h + 1])
                return at

            def ctx_acc(h, jc, ctxp, at):
                nc.tensor.matmul(ctxp[:], fr[:, jc, h, :], at[:],
                                 start=(jc == 0), stop=(jc == JC - 1))

            def head_recip(ctxp):
                rec = fin.tile([1, RN], F32, tag="rec")
                nc.vector.reciprocal(rec[:], ctxp[HD:HD + 1, :])
                return rec

            def head_finalize_b(h, ctxp, rec):
                recbp = psfr.tile([HD, RN], F32, tag="fr")
                nc.tensor.matmul(recbp[:], ones64[:], rec[:],
                                 start=True, stop=True)
                recs = fin.tile([HD, RN], F32, tag="recs")
                nc.vector.tensor_copy(recs[:], recbp[:])
                nc.vector.tensor_mul(
                    ctxT[(h % 2) * HD:(h % 2 + 1) * HD, h // 2, :],
                    ctxp[0:HD, :], recs[:])

            def head_finalize(h, ctxp, rec=None):
                if rec is None:
                    rec = head_recip(ctxp)
                head_finalize_b(h, ctxp, rec)

            # ---------- fused fr + sr + attention(heads 0..3) over jc -------
            ctxA = [psctx.tile([HD + 1, RN], F32, tag="ctx", name=f"ctxA{i}")
                    for i in range(HG)]

            def fr_matmuls(jc):
                frp = psfr.tile([128, D], F32, tag="fr")
                for dc in range(DC):
                    nc.tensor.matmul(frp[:], hT[:, dc, jc * 128:(jc + 1) * 128],
                                     wr[:, dc, :], start=(dc == 0),
                                     stop=(dc == DC - 1))
                return frp

            def sr_chain(jc, frp):
                # fr -> sbuf (bf16, strided over the ones column)
                nc.vector.tensor_copy(fr[:, jc, :, 0:HD],
                                      frp[:].rearrange("p (h d) -> p h d", h=H))
                # sr_j = sum_hd leaky(fr)*ar per head; leaky = max(x, .01x)
                lk = work.tile([128, D], BF, tag="lk")
                frv = fr[:, jc, :, 0:HD]
                nc.vector.scalar_tensor_tensor(
                    lk[:].rearrange("p (h d) -> p h d", h=H), frv, SLOPE, frv,
                    op0=mybir.AluOpType.mult, op1=mybir.AluOpType.max)
                lka = work.tile([128, D], BF, tag="lka")
                nc.vector.tensor_mul(lka[:], lk[:], arT[:])
                nc.vector.tensor_reduce(sr[:, jc, :],
                                        lka[:].rearrange("p (h d) -> p h d", h=H),
                                        mybir.AxisListType.X, mybir.AluOpType.add)

            frp_cur = fr_matmuls(0)
            for jc in range(JC):
                frp_next = fr_matmuls(jc + 1) if jc + 1 < JC else None
                sr_chain(jc, frp_cur)
                ats = [bias_exp(h, jc) for h in range(HG)]
                for h in range(HG):
                    ctx_acc(h, jc, ctxA[h], ats[h])
                frp_cur = frp_next

            # ---------- attention heads 4..7 (finalizes interleaved) -------
            pending = [(h, ctxA[h], head_recip(ctxA[h])) for h in range(HG)]

            def head_pair(hA, hB, carryA, carryB, next_heads):
                ctxpA = psctx.tile([HD + 1, RN], F32, tag="ctx",
                                   name=f"ctxB{hA}")
                ctxpB = psctx.tile([HD + 1, RN], F32, tag="ctx",
                                   name=f"ctxB{hB}")
                pA = carryA or bias_exp(hA, 0)
                pB = carryB or bias_exp(hB, 0)
                nca = ncb = None
                for jc in range(JC):
                    if jc + 1 < JC:
                        nA, nB = bias_exp(hA, jc + 1), bias_exp(hB, jc + 1)
                    else:
                        nA = nB = None
                        if next_heads:
                            nca = bias_exp(next_heads[0], 0)
                            if len(next_heads) > 1:
                                ncb = bias_exp(next_heads[1], 0)
                    nc.tensor.matmul(ctxpA[:], fr[:, jc, hA, :], pA[:],
                                     start=(jc == 0), stop=(jc == JC - 1))
                    nc.tensor.matmul(ctxpB[:], fr[:, jc, hB, :], pB[:],
                                     start=(jc == 0), stop=(jc == JC - 1))
                    pA, pB = nA, nB
                    if pending and jc % 3 == 2:
                        ph, pctx, prec = pending.pop(0)
                        head_finalize_b(ph, pctx, prec)
                pending.append((hA, ctxpA, head_recip(ctxpA)))
                pending.append((hB, ctxpB, head_recip(ctxpB)))
                return nca, ncb

            if H - HG == 3:
                ca, cb = head_pair(HG, HG + 1, None, None, [HG + 2])
                hC = HG + 2
                ctxpC = psctx.tile([HD + 1, RN], F32, tag="ctx", name="ctxBC")
                prev = ca
                for jc in range(JC):
                    nxt = bias_exp(hC, jc + 1) if jc + 1 < JC else None
                    nc.tensor.matmul(ctxpC[:], fr[:, jc, hC, :], prev[:],
                                     start=(jc == 0), stop=(jc == JC - 1))
                    prev = nxt
                    if pending and jc % 2 == 1:
                        ph, pctx, prec = pending.pop(0)
                        head_finalize_b(ph, pctx, prec)
                pending.append((hC, ctxpC, head_recip(ctxpC)))
            else:
                ca, cb = head_pair(HG, HG + 1, None, None, [HG + 2, HG + 3])
                head_pair(HG + 2, HG + 3, ca, cb, [])
            for ph, pctx, prec in pending:
                head_finalize_b(ph, pctx, prec)

            # ---------- fh + residual + LayerNorm ----------
            for ic in range(IC):
                fhp = ps.tile([128, D], F32, tag="pp")
                for t in range(DC):
                    nc.tensor.matmul(fhp[:],
                                     ctxT[:, t, ic * 128:(ic + 1) * 128],
                                     wf[:, t, :], start=(t == 0),
                                     stop=(t == DC - 1))
                x = fin.tile([128, D], F32, tag="x")
                nc.vector.tensor_add(x[:], hrow[:, ic, :], fhp[:])
                st = fin.tile([128, 6], F32, tag="st")
                nc.vector.bn_stats(st[:], x[:])
                mv = fin.tile([128, 2], F32, tag="mv")
                nc.vector.bn_aggr(mv[:], st[:])
                std = fin.tile([128, 1], F32, tag="std")
                nc.scalar.activation(std[:], mv[:, 1:2], SQRT, bias=epsc[:])
                rstd = fin.tile([128, 1], F32, tag="rstd")
                nc.vector.reciprocal(rstd[:], std[:])
                xm = fin.tile([128, D], F32, tag="xm")
                nc.vector.scalar_tensor_tensor(
                    xm[:], x[:], mv[:, 0:1], gam[:],
                    op0=mybir.AluOpType.subtract, op1=mybir.AluOpType.mult)
                xs = fin.tile([128, D], F32, tag="xs")
                nc.scalar.activation(xs[:], xm[:], COPY, scale=rstd[:])
                y = fin.tile([128, D], F32, tag="y")
                nc.gpsimd.tensor_add(y[:], xs[:], bet[:])
                nc.sync.dma_start(out_d[ic * 128:(ic + 1) * 128, :], y[:])

    nc.compile()
    return nc


def _get_graph():
    if "nc" not in _CACHE:
        _CACHE["nc"] = _build_graph()
    return _CACHE["nc"]


def _make_in_maps(h, rh, Wr, ar, Wrs, Wrt, Wf, gamma, beta):
    h = np.asarray(h, np.float32)
    rh = np.asarray(rh, np.float32)
    Wr_b = np.asarray(Wr, np.float32).astype(BF16)
    Wf_b = np.asarray(Wf, np.float32).astype(BF16)
    def _pack_groups(W):
        W = np.asarray(W, np.float32).reshape(RD, H, RD)
        P = np.zeros((RD, 3, 96), np.float32)
        for h in range(H):
            g, o = h // 3, (h % 3) * 32
            P[:, g, o:o + RD] = W[:, h, :]
        return P.astype(BF16)

    Wrs_b = _pack_groups(Wrs)
    Wrt_b = _pack_groups(Wrt)
    arT = np.ascontiguousarray(np.broadcast_to(np.tile(np.asarray(ar, np.float32), H), (128, D))).astype(BF16)
    gam = np.ascontiguousarray(np.broadcast_to(np.asarray(gamma, np.float32), (128, D)))
    bet = np.ascontiguousarray(np.broadcast_to(np.asarray(beta, np.float32), (128, D)))

    in_maps = []
    for c in range(NCORE):
        b, q = c // Q, c % Q
        rows = slice(q * RN, (q + 1) * RN)
        in_maps.append({
            "hT": np.ascontiguousarray(h[b].T).astype(BF16),
            "hrow": np.ascontiguousarray(h[b, rows, :]),
            "rhT": np.ascontiguousarray(rh[b].T).astype(BF16),
            "rhTq": np.ascontiguousarray(rh[b, rows, :].T).astype(BF16),
            "Wr": Wr_b, "Wf": Wf_b, "Wrs": Wrs_b, "Wrt": Wrt_b,
            "arT": arT, "gamma": gam, "beta": bet,
        })
    return in_maps


LAST_RESULT = {}


def kernel(h, rh, Wl, Wr, al, ar, Wrs, Wrt, Wf, gamma, beta,
           _trace=False):
    nc = _get_graph()
    in_maps = _make_in_maps(h, rh, Wr, ar, Wrs, Wrt, Wf, gamma, beta)
    for attempt in range(3):
        res = run_bass_kernel_spmd(nc, in_maps, list(range(NCORE)),
                                   trace=_trace)
        LAST_RESULT["res"] = res
        out = np.empty((B, N, D), np.float32)
        for c in range(NCORE):
            b, q = c // Q, c % Q
            out[b, q * RN:(q + 1) * RN, :] = res.results[c]["out"]
        if np.isfinite(out).all():
            return out
    return out
